# revision 1
# baseline (speedup 1.0000x reference)
"""Trainium2 Bass kernel for nn_Net_79465484911206: GRU(H=8) over x[4096,200,64] -> [4096].

Strategy (pure data parallel, 8 cores, 512 samples each; see sharding_hint):
- Host pre-arranges x per core as bf16 [128=(t%2 * 64 + f), 100=t//2, 4=chunk, 128=sample]
  so each (t, chunk) slice is a ready-made matmul *stationary* [64f, 128samples].
- Per time step, PSUM is born batch-on-partitions [128 samples, (4 chunks, 32 slots)]:
  slots per chunk: 0:8 r_pre, 8:16 z_pre, 16:24 ghn (w_hn h + b_hn), 24:32 xpn (w_in x + b_in).
  Accumulated by: 1 bias matmul (K=1 ones) + 4 x-matmuls (K=64) + 4 h-matmuls
  (K=32, block-transposed state, zero-padded rhs).
- Gates: sigmoid on ACT, r*ghn + xpn on DVE, tanh on ACT, 3 tensor ops for
  h' = z*(h-n) + n, then a 32x32 block transpose (DVE) regenerates h^T for the
  next step's stationary.
- Decode: h * w_dec reduce + b_dec, DMA out [128, 4] per core.

Self-contained: hardcodes all shapes; host does sharding + layout prep in numpy.
"""

import os
import numpy as np
import ml_dtypes

bf16 = ml_dtypes.bfloat16

B, T, F, H = 4096, 200, 64, 8
NCORES = 8
BL = B // NCORES          # 512 per core
NCH = 4                   # chunks of 128 samples
T2 = T // 2               # 100
DMA_T2 = 10               # t2-steps per x DMA chunk

LAST_RESULTS = None       # test.py reads exec_time_ns from here


def _build_program(b_dec_val: float):
    import concourse.bacc as bacc
    import concourse.mybir as mybir
    from concourse.tile import TileContext
    from concourse.tile_rust import add_dep_helper

    AF = mybir.ActivationFunctionType
    dt = mybir.dt

    nc = bacc.Bacc(
        "TRN2", target_bir_lowering=False, debug=False, num_devices=NCORES
    )

    x2_d = nc.dram_tensor("x2", [128, T2, NCH, 128], dt.bfloat16, kind="ExternalInput").ap()
    wihx_d = nc.dram_tensor("wihx", [128, 32], dt.bfloat16, kind="ExternalInput").ap()
    whhr_d = nc.dram_tensor("whhr", [128, 128], dt.bfloat16, kind="ExternalInput").ap()
    biasr_d = nc.dram_tensor("biasr", [1, 128], dt.bfloat16, kind="ExternalInput").ap()
    ones_d = nc.dram_tensor("ones", [1, 128], dt.bfloat16, kind="ExternalInput").ap()
    wdec_d = nc.dram_tensor("wdec", [128, 32], dt.bfloat16, kind="ExternalInput").ap()
    out_d = nc.dram_tensor("out", [128, NCH], dt.float32, kind="ExternalOutput").ap()

    with TileContext(nc) as tc:
        with (
            tc.tile_pool(name="consts", bufs=1) as cpool,
            tc.tile_pool(name="xin", bufs=2) as xpool,
            tc.tile_pool(name="state", bufs=1) as spool,
            tc.tile_pool(name="work", bufs=3) as wpool,
            tc.tile_pool(name="gates", bufs=4, space="PSUM") as gpool,
        ):
            # constants
            wihx = cpool.tile([128, 32], dt.bfloat16)
            nc.sync.dma_start(out=wihx[:], in_=wihx_d)
            whhr = cpool.tile([128, 128], dt.bfloat16)
            nc.sync.dma_start(out=whhr[:], in_=whhr_d)
            biasr = cpool.tile([1, 128], dt.bfloat16)
            nc.sync.dma_start(out=biasr[:], in_=biasr_d)
            ones = cpool.tile([1, 128], dt.bfloat16)
            nc.sync.dma_start(out=ones[:], in_=ones_d)
            wdec = cpool.tile([128, 32], dt.bfloat16)
            nc.sync.dma_start(out=wdec[:], in_=wdec_d)

            # state: h [128, (4, 8)] bf16 and its 32x32 block transpose hT
            h = spool.tile([128, 32], dt.bfloat16)
            nc.vector.memset(h[:], 0.0)
            hT = spool.tile([128, 32], dt.bfloat16)
            nc.vector.memset(hT[:], 0.0)

            xsb = None
            xsb_map = {}
            ps_tiles = {}

            AB = os.environ.get("OPT_ABLATE", "")

            def emit_mmx(t):
                t2, tp = divmod(t, 2)
                ki, ko = divmod(t2, DMA_T2)
                psA = gpool.tile([128, 96], dt.float32, tag="psA", name=f"psA{t}")
                psB = gpool.tile([128, 32], dt.float32, tag="psB", name=f"psB{t}")
                xsb = xsb_map[ki]
                bias_a = nc.tensor.matmul(
                    psA[:], ones[:], biasr[:, 0:96], start=True, stop=False,
                    skip_group_check=True)
                bias_b = nc.tensor.matmul(
                    psB[:], ones[:], biasr[:, 96:128], start=True, stop=False,
                    skip_group_check=True)
                xas, xbs = [], []
                for c in range(NCH):
                    xas.append(nc.tensor.matmul(
                        psA[:, 24 * c:24 * c + 24],
                        xsb[tp * 64:(tp + 1) * 64, ko, c, :],
                        wihx[tp * 64:(tp + 1) * 64, 0:24],
                        start=False, stop=False, skip_group_check=True))
                    xbs.append(nc.tensor.matmul(
                        psB[:, 8 * c:8 * c + 8],
                        xsb[tp * 64:(tp + 1) * 64, ko, c, :],
                        wihx[tp * 64:(tp + 1) * 64, 24:32],
                        start=False, stop=(c == NCH - 1), skip_group_check=True))
                for m in xas:
                    add_dep_helper(m.ins, bias_a.ins, False, "accum order")
                for m in xbs:
                    add_dep_helper(m.ins, bias_b.ins, False, "accum order")
                ps_tiles[t] = (psA, psB, xas)

            def emit_step(t):
                psA, psB, xas = ps_tiles[t]
                rz = wpool.tile([128, NCH, 16], dt.bfloat16, tag="rz", name="rz")
                m1 = wpool.tile([128, NCH, 8], dt.float32, tag="m1", name="m1")
                u = wpool.tile([128, NCH, 8], dt.float32, tag="u", name="u")
                n = wpool.tile([128, NCH, 8], dt.bfloat16, tag="n", name="n")
                y = wpool.tile([128, NCH, 8], dt.bfloat16, tag="y", name="y")
                zh = wpool.tile([128, NCH, 8], dt.bfloat16, tag="zh", name="zh")
                ny = wpool.tile([128, NCH, 8], dt.bfloat16, tag="ny", name="ny")
                gh = psA[:].rearrange("p (c g) -> p c g", c=NCH)
                xpn = psB[:].rearrange("p (c j) -> p c j", c=NCH)
                hv = h[:].rearrange("p (c j) -> p c j", c=NCH)
                for i in range(NCH):
                    hm = nc.tensor.matmul(
                        psA[32 * i:32 * (i + 1), :],
                        hT[32 * i:32 * (i + 1), :],
                        whhr[32 * i:32 * (i + 1), 0:96],
                        start=False, stop=(i == NCH - 1), skip_group_check=True,
                        tile_position=(32 * i, 32 * i),
                    )
                    for m in xas:
                        add_dep_helper(hm.ins, m.ins, False, "accum order")
                nc.scalar.activation(rz[:], gh[:, :, 0:16], AF.Sigmoid)
                # off-chain: y = 1 - z, zh = z * h (h is still the old state)
                nc.vector.tensor_scalar(
                    y[:], rz[:, :, 8:16], -1.0, 1.0,
                    mybir.AluOpType.mult, mybir.AluOpType.add)
                nc.vector.tensor_mul(zh[:], rz[:, :, 8:16], hv)
                nc.vector.tensor_mul(m1[:], rz[:, :, 0:8], gh[:, :, 16:24])
                nc.vector.tensor_add(u[:], m1[:], xpn)
                nc.scalar.activation(n[:], u[:], AF.Tanh)
                # on-chain tail: h' = n*(1-z) + z*h
                nc.vector.tensor_mul(ny[:], n[:], y[:])
                nc.vector.tensor_add(hv, ny[:], zh[:])
                nc.vector.transpose(hT[:], h[:])

            for t in range(T):
                t2, tp = divmod(t, 2)
                ki, ko = divmod(t2, DMA_T2)
                if ko == 0 and tp == 0:
                    xsb = xpool.tile([128, DMA_T2, NCH, 128], dt.bfloat16,
                                     tag="xsb", name=f"xsb{ki}")
                    nc.sync.dma_start(
                        out=xsb[:],
                        in_=x2_d[:, ki * DMA_T2:(ki + 1) * DMA_T2, :, :],
                    )
                    xsb_map[ki] = xsb
                emit_mmx(t)
                emit_step(t)
                ps_tiles.pop(t, None)

            # decode: out[p, c] = sum_j h * wdec + b_dec
            prod = wpool.tile([128, NCH, 8], dt.float32, tag="prod")
            nc.vector.tensor_mul(
                prod[:], h[:].rearrange("p (c j) -> p c j", c=NCH),
                wdec[:].rearrange("p (c j) -> p c j", c=NCH),
            )
            res = wpool.tile([128, NCH, 1], dt.float32, tag="res")
            nc.vector.tensor_reduce(
                res[:], prod[:], axis=mybir.AxisListType.X, op=mybir.AluOpType.add
            )
            res2 = wpool.tile([128, NCH], dt.float32, tag="res2")
            nc.vector.tensor_scalar_add(
                res2[:], res[:].rearrange("p c one -> p (c one)"), float(b_dec_val))
            nc.sync.dma_start(out=out_d, in_=res2[:])

    nc.compile()
    return nc


def _prep_inputs(x, w_ih, w_hh, b_ih, b_hh, w_dec, b_dec):
    """Returns per-core in_maps list."""
    w_ih = np.asarray(w_ih, np.float32)
    w_hh = np.asarray(w_hh, np.float32)
    b_ih = np.asarray(b_ih, np.float32)
    b_hh = np.asarray(b_hh, np.float32)
    w_dec = np.asarray(w_dec, np.float32)

    wihx = np.zeros((64, 32), np.float32)
    wihx[:, 0:8] = w_ih[0:8].T
    wihx[:, 8:16] = w_ih[8:16].T
    wihx[:, 24:32] = w_ih[16:24].T   # cols 0:24 rzn-slot, 24:32 xpn
    wihx = np.tile(wihx, (2, 1)).astype(bf16)

    whhr = np.zeros((32, NCH, 24), np.float32)
    for c in range(NCH):
        # rhs[(c,j), c, g] = w_hh[g, j]
        whhr[c * 8:(c + 1) * 8, c, :] = w_hh.T  # [8j, 24g]
    whhr = whhr.reshape(32, 96)
    whhr = np.concatenate([whhr, np.zeros((32, 32), np.float32)], axis=1)
    whhr = np.tile(whhr, (4, 1)).astype(bf16)

    bias24 = np.concatenate([b_ih[0:8] + b_hh[0:8],
                             b_ih[8:16] + b_hh[8:16],
                             b_hh[16:24]])
    bias8 = b_ih[16:24]
    biasr = np.concatenate([np.tile(bias24, NCH), np.tile(bias8, NCH)])
    biasr = biasr.reshape(1, 128).astype(bf16)

    ones = np.ones((1, 128), bf16)
    wdec_b = np.tile(w_dec[0].astype(bf16).astype(np.float32), (128, NCH)).astype(bf16)

    x = np.asarray(x, np.float32)
    in_maps = []
    for core in range(NCORES):
        xc = x[core * BL:(core + 1) * BL]                      # [512, 200, 64]
        tmp = xc.reshape(NCH, 128, T2, 2, 64)                  # ch, s, t2, tp, f
        x2 = np.ascontiguousarray(
            tmp.transpose(3, 4, 2, 0, 1).reshape(128, T2, NCH, 128)
        ).astype(bf16)
        in_maps.append({
            "x2": x2, "wihx": wihx, "whhr": whhr, "biasr": biasr,
            "ones": ones, "wdec": wdec_b,
        })
    return in_maps


def kernel(x, w_ih, w_hh, b_ih, b_hh, w_dec, b_dec):
    global LAST_RESULTS
    from concourse import bass_utils

    b_dec_val = float(np.asarray(b_dec, np.float32).reshape(-1)[0])
    nc = _build_program(b_dec_val)
    in_maps = _prep_inputs(x, w_ih, w_hh, b_ih, b_hh, w_dec, b_dec)
    res = bass_utils.run_bass_kernel_spmd(
        nc, in_maps, core_ids=list(range(NCORES)),
        trace=bool(int(os.environ.get("KERNEL_TRACE", "0"))),
    )
    LAST_RESULTS = res
    out = np.empty(B, np.float32)
    for core in range(NCORES):
        o = np.asarray(res.results[core]["out"])               # [128, 4]
        out[core * BL:(core + 1) * BL] = o.T.reshape(-1)
    return out



# revision 5
# speedup vs baseline: 1.0940x; 1.0940x over previous
"""Trainium2 Bass kernel for nn_Net_79465484911206: GRU(H=8) over x[4096,200,64] -> [4096].

Data parallel across 8 cores (512 samples each, as 4 chunks of 128 on
partitions).  The per-step dependency cycle is the bottleneck (engines are
~75% idle), so the kernel is organized to minimize the serial chain:

- h' = (1-z)*n + z*h is never materialized on the critical path.  Instead
  the recurrent matmul W_hh @ h' is split into W_hh @ zh (ready early, off
  the chain) and W_hh @ ny (the only late dependency), using transposed
  bf16 stationaries zhT / nyT.
- PSUM is split per gate group: ps_r (r pre-acts), ps_zn (z pre-acts +
  W_hn h part for n), ps_xpn (x-only n projection).  sigmoid(r) waits only
  on the tiny mm_ny_r matmuls (ap=32), not the full 96-col update.
- ps_xpn is complete long before the chain needs it, so an off-chain ACT
  copy stages it in SBUF and u = m1 + xpn avoids a second PSUM access.
- Off-chain DVE work (y, zh, zhT, h') is ordered after the on-chain ops
  (m1, u) so the in-order DVE queue never delays the chain.

Critical cycle per step:
  mm_ny_r (PE) -> sigmoid_r (ACT) -> m1 (DVE) -> u (DVE) -> tanh (ACT)
  -> ny (DVE) -> nyT (DVE) -> mm_ny (PE, next step)

Self-contained: hardcodes all shapes; host does sharding + layout prep.
"""

import os
import numpy as np
import ml_dtypes

bf16 = ml_dtypes.bfloat16

B, T, F, H = 4096, 200, 64, 8
NCORES = 8
BL = B // NCORES          # 512 per core
NCH = 4                   # chunks of 128 samples
T2 = T // 2               # 100
DMA_T2 = 10               # t2-steps per x DMA chunk

LAST_RESULTS = None       # test.py reads exec_time_ns from here


def _build_program(b_dec_val: float):
    import concourse.bacc as bacc
    import concourse.mybir as mybir
    from concourse.tile import TileContext
    from concourse.tile_rust import add_dep_helper

    AF = mybir.ActivationFunctionType
    ALU = mybir.AluOpType
    dt = mybir.dt

    nc = bacc.Bacc(
        "TRN2", target_bir_lowering=False, debug=False, num_devices=NCORES
    )

    x2_d = nc.dram_tensor("x2", [128, T2, NCH, 128], dt.bfloat16, kind="ExternalInput").ap()
    wihr_d = nc.dram_tensor("wihr", [128, 8], dt.bfloat16, kind="ExternalInput").ap()
    wihzn_d = nc.dram_tensor("wihzn", [128, 16], dt.bfloat16, kind="ExternalInput").ap()
    wihn_d = nc.dram_tensor("wihn", [128, 8], dt.bfloat16, kind="ExternalInput").ap()
    whhr_d = nc.dram_tensor("whhr", [128, 32], dt.bfloat16, kind="ExternalInput").ap()
    whhzn_d = nc.dram_tensor("whhzn", [128, 64], dt.bfloat16, kind="ExternalInput").ap()
    biasr_d = nc.dram_tensor("biasr", [1, 32], dt.bfloat16, kind="ExternalInput").ap()
    biaszn_d = nc.dram_tensor("biaszn", [1, 64], dt.bfloat16, kind="ExternalInput").ap()
    biasn_d = nc.dram_tensor("biasn", [1, 32], dt.bfloat16, kind="ExternalInput").ap()
    ones_d = nc.dram_tensor("ones", [1, 128], dt.bfloat16, kind="ExternalInput").ap()
    wdec_d = nc.dram_tensor("wdec", [128, 32], dt.bfloat16, kind="ExternalInput").ap()
    out_d = nc.dram_tensor("out", [128, NCH], dt.float32, kind="ExternalOutput").ap()

    with TileContext(nc) as tc:
        with (
            tc.tile_pool(name="consts", bufs=1) as cpool,
            tc.tile_pool(name="xin", bufs=2) as xpool,
            tc.tile_pool(name="state", bufs=1) as spool,
            tc.tile_pool(name="work", bufs=3) as wpool,
            tc.tile_pool(name="psr", bufs=2, space="PSUM") as prpool,
            tc.tile_pool(name="pszn", bufs=2, space="PSUM") as znpool,
            tc.tile_pool(name="psx", bufs=2, space="PSUM") as xppool,
        ):
            # constants
            wihr = cpool.tile([128, 8], dt.bfloat16)
            nc.sync.dma_start(out=wihr[:], in_=wihr_d)
            wihzn = cpool.tile([128, 16], dt.bfloat16)
            nc.sync.dma_start(out=wihzn[:], in_=wihzn_d)
            wihn = cpool.tile([128, 8], dt.bfloat16)
            nc.sync.dma_start(out=wihn[:], in_=wihn_d)
            whhr = cpool.tile([128, 32], dt.bfloat16)
            nc.sync.dma_start(out=whhr[:], in_=whhr_d)
            whhzn = cpool.tile([128, 64], dt.bfloat16)
            nc.sync.dma_start(out=whhzn[:], in_=whhzn_d)
            biasr = cpool.tile([1, 32], dt.bfloat16)
            nc.sync.dma_start(out=biasr[:], in_=biasr_d)
            biaszn = cpool.tile([1, 64], dt.bfloat16)
            nc.sync.dma_start(out=biaszn[:], in_=biaszn_d)
            biasn = cpool.tile([1, 32], dt.bfloat16)
            nc.sync.dma_start(out=biasn[:], in_=biasn_d)
            ones = cpool.tile([1, 128], dt.bfloat16)
            nc.sync.dma_start(out=ones[:], in_=ones_d)
            wdec = cpool.tile([128, 32], dt.bfloat16)
            nc.sync.dma_start(out=wdec[:], in_=wdec_d)

            # state: h [128, (4, 8)] bf16; zhT/nyT transposed update parts
            h = spool.tile([128, 32], dt.bfloat16)
            nc.vector.memset(h[:], 0.0)
            zhT = spool.tile([128, 32], dt.bfloat16)
            nc.vector.memset(zhT[:], 0.0)
            nyT = spool.tile([128, 32], dt.bfloat16)
            nc.vector.memset(nyT[:], 0.0)

            xsb = None
            xsb_map = {}
            ps_map = {}

            def emit_x(t):
                """bias + x matmuls for step t (everything h-independent)."""
                t2, tp = divmod(t, 2)
                ki, ko = divmod(t2, DMA_T2)
                xsb = xsb_map[ki]
                psr = prpool.tile([128, NCH, 8], dt.float32, tag="psr", name=f"psr{t}")
                pszn = znpool.tile([128, NCH, 16], dt.float32, tag="pszn", name=f"pszn{t}")
                psx = xppool.tile([128, NCH, 8], dt.float32, tag="psx", name=f"psx{t}")
                psr2 = psr[:].rearrange("p c g -> p (c g)")
                pszn2 = pszn[:].rearrange("p c g -> p (c g)")
                psx2 = psx[:].rearrange("p c g -> p (c g)")
                b_r = nc.tensor.matmul(psr2, ones[:], biasr[:], start=True, stop=False,
                                       skip_group_check=True)
                b_zn = nc.tensor.matmul(pszn2, ones[:], biaszn[:], start=True, stop=False,
                                        skip_group_check=True)
                b_x = nc.tensor.matmul(psx2, ones[:], biasn[:], start=True, stop=False,
                                       skip_group_check=True)
                xs = []
                for c in range(NCH):
                    stat = xsb[tp * 64:(tp + 1) * 64, ko, c, :]
                    m_r = nc.tensor.matmul(psr[:, c, :], stat, wihr[tp * 64:(tp + 1) * 64, :],
                                           start=False, stop=False, skip_group_check=True)
                    m_z = nc.tensor.matmul(pszn[:, c, :], stat, wihzn[tp * 64:(tp + 1) * 64, :],
                                           start=False, stop=False, skip_group_check=True)
                    m_x = nc.tensor.matmul(psx[:, c, :], stat, wihn[tp * 64:(tp + 1) * 64, :],
                                           start=False, stop=(c == NCH - 1),
                                           skip_group_check=True)
                    add_dep_helper(m_r.ins, b_r.ins, False, "accum order")
                    add_dep_helper(m_z.ins, b_zn.ins, False, "accum order")
                    add_dep_helper(m_x.ins, b_x.ins, False, "accum order")
                    xs.append((m_r, m_z))
                ps_map[t] = (psr, pszn, psx, xs)

            def emit_hmm(t, statT, stop, after=None):
                """4 block matmuls of W_hh against stationary statT (zhT or nyT)."""
                psr, pszn, psx, xs = ps_map[t]
                psr2 = psr[:].rearrange("p c g -> p (c g)")
                pszn2 = pszn[:].rearrange("p c g -> p (c g)")
                mms = []
                for i in range(NCH):
                    last = stop and (i == NCH - 1)
                    mr = nc.tensor.matmul(
                        psr2[32 * i:32 * (i + 1), :],
                        statT[32 * i:32 * (i + 1), :],
                        whhr[32 * i:32 * (i + 1), :],
                        start=False, stop=last, skip_group_check=True,
                        tile_position=(32 * i, 32 * i))
                    mz = nc.tensor.matmul(
                        pszn2[32 * i:32 * (i + 1), :],
                        statT[32 * i:32 * (i + 1), :],
                        whhzn[32 * i:32 * (i + 1), :],
                        start=False, stop=last,
                        skip_group_check=True,
                        tile_position=(32 * i, 32 * i))
                    for (m_r, m_z) in xs:
                        add_dep_helper(mr.ins, m_r.ins, False, "accum order")
                        add_dep_helper(mz.ins, m_z.ins, False, "accum order")
                    if after is not None:
                        pr, pz = after[i]
                        add_dep_helper(mr.ins, pr.ins, False, "accum order")
                        add_dep_helper(mz.ins, pz.ins, False, "accum order")
                    mms.append((mr, mz))
                return mms

            def emit_step(t):
                psr, pszn, psx, xs = ps_map[t]
                r = wpool.tile([128, NCH, 8], dt.bfloat16, tag="r", name="r")
                z = wpool.tile([128, NCH, 8], dt.bfloat16, tag="z", name="z")
                xpn = wpool.tile([128, NCH, 8], dt.float32, tag="xpn", name="xpn")
                m1 = wpool.tile([128, NCH, 8], dt.float32, tag="m1", name="m1")
                u = wpool.tile([128, NCH, 8], dt.float32, tag="u", name="u")
                n = wpool.tile([128, NCH, 8], dt.bfloat16, tag="n", name="n")
                y = wpool.tile([128, NCH, 8], dt.bfloat16, tag="y", name="y")
                zh = wpool.tile([128, NCH, 8], dt.bfloat16, tag="zh", name="zh")
                ny = wpool.tile([128, NCH, 8], dt.bfloat16, tag="ny", name="ny")

                # ACT: xpn staging copy (off-chain; psx closed early), then
                # on-chain sigmoid_r, off-chain sigmoid_z, on-chain tanh.
                nc.scalar.copy(xpn[:], psx[:])
                nc.scalar.activation(r[:], psr[:], AF.Sigmoid)
                nc.scalar.activation(z[:], pszn[:, :, 0:8], AF.Sigmoid)

                # DVE on-chain: m1 = r*ghn (single PSUM read), u = m1 + xpn
                nc.vector.tensor_mul(m1[:], r[:], pszn[:, :, 8:16])
                nc.vector.tensor_add(u[:], m1[:], xpn[:])

                nc.scalar.activation(n[:], u[:], AF.Tanh)

                # DVE off-chain (fills the u->tanh window): y, zh, zhT
                nc.vector.tensor_scalar(y[:], z[:], -1.0, 1.0, ALU.mult, ALU.add)
                hv = h[:].rearrange("p (c j) -> p c j", c=NCH)
                nc.vector.tensor_mul(zh[:], z[:], hv)
                nc.vector.transpose(zhT[:], zh[:].rearrange("p c j -> p (c j)"))

                # DVE on-chain tail: ny, nyT
                nc.vector.tensor_mul(ny[:], n[:], y[:])
                nc.vector.transpose(nyT[:], ny[:].rearrange("p c j -> p (c j)"))

                # DVE off-chain: h' = ny + zh (for next zh and final decode)
                nc.vector.tensor_add(hv, ny[:], zh[:])

            for t in range(T):
                t2, tp = divmod(t, 2)
                ki, ko = divmod(t2, DMA_T2)
                if ko == 0 and tp == 0:
                    xsb = xpool.tile([128, DMA_T2, NCH, 128], dt.bfloat16,
                                     tag="xsb", name=f"xsb{ki}")
                    nc.sync.dma_start(
                        out=xsb[:],
                        in_=x2_d[:, ki * DMA_T2:(ki + 1) * DMA_T2, :, :],
                    )
                    xsb_map[ki] = xsb
                emit_x(t)
                # W_hh @ zh(t-1): ready early, off the critical chain
                # (at t=0 both stationaries are the zero-memset tiles: exact)
                zh_mms = emit_hmm(t, zhT, stop=False)
                # W_hh @ ny(t-1): the critical-path matmuls
                emit_hmm(t, nyT, stop=True, after=zh_mms)
                emit_step(t)
                ps_map.pop(t - 2, None)

            # decode: out[p, c] = sum_j h * wdec + b_dec
            prod = wpool.tile([128, NCH, 8], dt.float32, tag="prod")
            nc.vector.tensor_mul(
                prod[:], h[:].rearrange("p (c j) -> p c j", c=NCH),
                wdec[:].rearrange("p (c j) -> p c j", c=NCH),
            )
            res = wpool.tile([128, NCH, 1], dt.float32, tag="res")
            nc.vector.tensor_reduce(
                res[:], prod[:], axis=mybir.AxisListType.X, op=mybir.AluOpType.add
            )
            res2 = wpool.tile([128, NCH], dt.float32, tag="res2")
            nc.vector.tensor_scalar_add(
                res2[:], res[:].rearrange("p c one -> p (c one)"), float(b_dec_val))
            nc.sync.dma_start(out=out_d, in_=res2[:])

    nc.compile()
    return nc


def _prep_inputs(x, w_ih, w_hh, b_ih, b_hh, w_dec, b_dec):
    """Returns per-core in_maps list."""
    w_ih = np.asarray(w_ih, np.float32)
    w_hh = np.asarray(w_hh, np.float32)
    b_ih = np.asarray(b_ih, np.float32)
    b_hh = np.asarray(b_hh, np.float32)
    w_dec = np.asarray(w_dec, np.float32)

    # x-projection weights, stationary = x^T [64f, 128s], moving = wih*
    wihr = np.tile(w_ih[0:8].T, (2, 1)).astype(bf16)            # [128, 8]
    wihzn = np.zeros((64, 16), np.float32)
    wihzn[:, 0:8] = w_ih[8:16].T                                # z cols
    wihzn = np.tile(wihzn, (2, 1)).astype(bf16)                 # [128, 16]
    wihn = np.tile(w_ih[16:24].T, (2, 1)).astype(bf16)          # [128, 8]

    # recurrent weights, block-diag over chunks; stationary = (zh|ny)^T
    def blockdiag(wpart, gw):
        # wpart: [gw, 8] rows of w_hh ; returns [128, NCH*gw]
        m = np.zeros((32, NCH, gw), np.float32)
        for c in range(NCH):
            m[c * 8:(c + 1) * 8, c, :] = wpart.T                # [8j, gw]
        m = m.reshape(32, NCH * gw)
        return np.tile(m, (4, 1)).astype(bf16)

    whhr = blockdiag(w_hh[0:8], 8)                              # [128, 32]
    whhzn = blockdiag(np.concatenate([w_hh[8:16], w_hh[16:24]]), 16)  # [128, 64]

    biasr = np.tile(b_ih[0:8] + b_hh[0:8], NCH).reshape(1, 32).astype(bf16)
    biaszn = np.tile(np.concatenate([b_ih[8:16] + b_hh[8:16], b_hh[16:24]]),
                     NCH).reshape(1, 64).astype(bf16)
    biasn = np.tile(b_ih[16:24], NCH).reshape(1, 32).astype(bf16)

    ones = np.ones((1, 128), bf16)
    wdec_b = np.tile(w_dec[0].astype(bf16).astype(np.float32), (128, NCH)).astype(bf16)

    x = np.asarray(x, np.float32)
    in_maps = []
    for core in range(NCORES):
        xc = x[core * BL:(core + 1) * BL]                      # [512, 200, 64]
        tmp = xc.reshape(NCH, 128, T2, 2, 64)                  # ch, s, t2, tp, f
        x2 = np.ascontiguousarray(
            tmp.transpose(3, 4, 2, 0, 1).reshape(128, T2, NCH, 128)
        ).astype(bf16)
        in_maps.append({
            "x2": x2, "wihr": wihr, "wihzn": wihzn, "wihn": wihn,
            "whhr": whhr, "whhzn": whhzn, "biasr": biasr, "biaszn": biaszn,
            "biasn": biasn, "ones": ones, "wdec": wdec_b,
        })
    return in_maps


def kernel(x, w_ih, w_hh, b_ih, b_hh, w_dec, b_dec):
    global LAST_RESULTS
    from concourse import bass_utils

    b_dec_val = float(np.asarray(b_dec, np.float32).reshape(-1)[0])
    nc = _build_program(b_dec_val)
    in_maps = _prep_inputs(x, w_ih, w_hh, b_ih, b_hh, w_dec, b_dec)
    res = bass_utils.run_bass_kernel_spmd(
        nc, in_maps, core_ids=list(range(NCORES)),
        trace=bool(int(os.environ.get("KERNEL_TRACE", "0"))),
    )
    LAST_RESULTS = res
    out = np.empty(B, np.float32)
    for core in range(NCORES):
        o = np.asarray(res.results[core]["out"])               # [128, 4]
        out[core * BL:(core + 1) * BL] = o.T.reshape(-1)
    return out


# revision 9
# speedup vs baseline: 1.1297x; 1.0326x over previous
"""Trainium2 Bass kernel for nn_Net_79465484911206: GRU(H=8) over x[4096,200,64] -> [4096].

Data parallel across 8 cores (512 samples each, as 4 chunks of 128 on
partitions).  The per-step dependency cycle is the bottleneck (engines are
~75% idle), so the kernel is organized to minimize the serial chain:

- h' = (1-z)*n + z*h is never materialized on the critical path.  Instead
  the recurrent matmul W_hh @ h' is split into W_hh @ zh (ready early, off
  the chain) and W_hh @ ny (the only late dependency), using transposed
  bf16 stationaries zhT / nyT.
- PSUM is split per gate group: ps_r (r pre-acts), ps_zn (z pre-acts +
  W_hn h part for n), ps_xpn (x-only n projection).  sigmoid(r) waits only
  on the tiny mm_ny_r matmuls (ap=32), not the full 96-col update.
- ps_xpn is complete long before the chain needs it, so an off-chain ACT
  copy stages it in SBUF and u = m1 + xpn avoids a second PSUM access.
- Off-chain DVE work (y, zh, zhT, h') is ordered after the on-chain ops
  (m1, u) so the in-order DVE queue never delays the chain.

Critical cycle per step:
  mm_ny_r (PE) -> sigmoid_r (ACT) -> m1 (DVE) -> u (DVE) -> tanh (ACT)
  -> ny (DVE) -> nyT (DVE) -> mm_ny (PE, next step)

Self-contained: hardcodes all shapes; host does sharding + layout prep.
"""

import os
import numpy as np
import ml_dtypes

bf16 = ml_dtypes.bfloat16

B, T, F, H = 4096, 200, 64, 8
NCORES = 8
BL = B // NCORES          # 512 per core
NCH = 4                   # chunks of 128 samples
T2 = T // 2               # 100
DMA_T2 = 10               # t2-steps per x DMA chunk

LAST_RESULTS = None       # test.py reads exec_time_ns from here


def _build_program(b_dec_val: float):
    import concourse.bacc as bacc
    import concourse.mybir as mybir
    from concourse.tile import TileContext
    from concourse.tile_rust import add_dep_helper

    AF = mybir.ActivationFunctionType
    ALU = mybir.AluOpType
    dt = mybir.dt

    nc = bacc.Bacc(
        "TRN2", target_bir_lowering=False, debug=False, num_devices=NCORES
    )

    x2_d = nc.dram_tensor("x2", [128, T2, NCH, 128], dt.bfloat16, kind="ExternalInput").ap()
    wihr_d = nc.dram_tensor("wihr", [128, 8], dt.bfloat16, kind="ExternalInput").ap()
    wihzn_d = nc.dram_tensor("wihzn", [128, 16], dt.bfloat16, kind="ExternalInput").ap()
    wihn_d = nc.dram_tensor("wihn", [128, 8], dt.bfloat16, kind="ExternalInput").ap()
    whhr_d = nc.dram_tensor("whhr", [128, 32], dt.bfloat16, kind="ExternalInput").ap()
    whhzn_d = nc.dram_tensor("whhzn", [128, 64], dt.bfloat16, kind="ExternalInput").ap()
    biasr_d = nc.dram_tensor("biasr", [1, 32], dt.bfloat16, kind="ExternalInput").ap()
    biaszn_d = nc.dram_tensor("biaszn", [1, 64], dt.bfloat16, kind="ExternalInput").ap()
    biasn_d = nc.dram_tensor("biasn", [1, 32], dt.bfloat16, kind="ExternalInput").ap()
    ones_d = nc.dram_tensor("ones", [1, 128], dt.bfloat16, kind="ExternalInput").ap()
    wdec_d = nc.dram_tensor("wdec", [128, 32], dt.bfloat16, kind="ExternalInput").ap()
    out_d = nc.dram_tensor("out", [128, NCH], dt.float32, kind="ExternalOutput").ap()

    with TileContext(nc) as tc:
        with (
            tc.tile_pool(name="consts", bufs=1) as cpool,
            tc.tile_pool(name="xin", bufs=2) as xpool,
            tc.tile_pool(name="state", bufs=1) as spool,
            tc.tile_pool(name="work", bufs=3) as wpool,
            tc.tile_pool(name="psr", bufs=2, space="PSUM") as prpool,
            tc.tile_pool(name="pszn", bufs=2, space="PSUM") as znpool,
            tc.tile_pool(name="psx", bufs=2, space="PSUM") as xppool,
        ):
            # constants
            wihr = cpool.tile([128, 8], dt.bfloat16)
            nc.sync.dma_start(out=wihr[:], in_=wihr_d)
            wihzn = cpool.tile([128, 16], dt.bfloat16)
            nc.sync.dma_start(out=wihzn[:], in_=wihzn_d)
            wihn = cpool.tile([128, 8], dt.bfloat16)
            nc.sync.dma_start(out=wihn[:], in_=wihn_d)
            whhr = cpool.tile([128, 32], dt.bfloat16)
            nc.sync.dma_start(out=whhr[:], in_=whhr_d)
            whhzn = cpool.tile([128, 64], dt.bfloat16)
            nc.sync.dma_start(out=whhzn[:], in_=whhzn_d)
            biasr = cpool.tile([1, 32], dt.bfloat16)
            nc.sync.dma_start(out=biasr[:], in_=biasr_d)
            biaszn = cpool.tile([1, 64], dt.bfloat16)
            nc.sync.dma_start(out=biaszn[:], in_=biaszn_d)
            biasn = cpool.tile([1, 32], dt.bfloat16)
            nc.sync.dma_start(out=biasn[:], in_=biasn_d)
            ones = cpool.tile([1, 128], dt.bfloat16)
            nc.sync.dma_start(out=ones[:], in_=ones_d)
            wdec = cpool.tile([128, 32], dt.bfloat16)
            nc.sync.dma_start(out=wdec[:], in_=wdec_d)

            # state: h [128, (4, 8)] bf16; zhT/nyT transposed update parts
            h = spool.tile([128, 32], dt.bfloat16)
            nc.vector.memset(h[:], 0.0)
            zhT = spool.tile([128, 32], dt.bfloat16)
            nc.vector.memset(zhT[:], 0.0)
            nyT = spool.tile([128, 32], dt.bfloat16)
            nc.vector.memset(nyT[:], 0.0)

            xsb = None
            xsb_map = {}
            ps_map = {}
            prev_tanh = [None]

            def emit_x(t):
                """bias + x matmuls for step t (everything h-independent)."""
                t2, tp = divmod(t, 2)
                ki, ko = divmod(t2, DMA_T2)
                xsb = xsb_map[ki]
                psr = prpool.tile([128, NCH, 8], dt.float32, tag="psr", name=f"psr{t}")
                pszn = znpool.tile([128, NCH, 16], dt.float32, tag="pszn", name=f"pszn{t}")
                psx = xppool.tile([128, NCH, 8], dt.float32, tag="psx", name=f"psx{t}")
                psr2 = psr[:].rearrange("p c g -> p (c g)")
                pszn2 = pszn[:].rearrange("p c g -> p (c g)")
                psx2 = psx[:].rearrange("p c g -> p (c g)")
                b_r = nc.tensor.matmul(psr2, ones[:], biasr[:], start=True, stop=False,
                                       skip_group_check=True)
                b_zn = nc.tensor.matmul(pszn2, ones[:], biaszn[:], start=True, stop=False,
                                        skip_group_check=True)
                b_x = nc.tensor.matmul(psx2, ones[:], biasn[:], start=True, stop=False,
                                       skip_group_check=True)
                xs = []
                for c in range(NCH):
                    stat = xsb[tp * 64:(tp + 1) * 64, ko, c, :]
                    m_r = nc.tensor.matmul(psr[:, c, :], stat, wihr[tp * 64:(tp + 1) * 64, :],
                                           start=False, stop=False, skip_group_check=True)
                    m_z = nc.tensor.matmul(pszn[:, c, :], stat, wihzn[tp * 64:(tp + 1) * 64, :],
                                           start=False, stop=False, skip_group_check=True)
                    m_x = nc.tensor.matmul(psx[:, c, :], stat, wihn[tp * 64:(tp + 1) * 64, :],
                                           start=False, stop=(c == NCH - 1),
                                           skip_group_check=True)
                    add_dep_helper(m_r.ins, b_r.ins, False, "accum order")
                    add_dep_helper(m_z.ins, b_zn.ins, False, "accum order")
                    add_dep_helper(m_x.ins, b_x.ins, False, "accum order")
                    xs.append((m_r, m_z))
                ps_map[t] = (psr, pszn, psx, xs)

            def emit_hmm(t, statT, stop, after=None):
                """Block matmuls of W_hh against stationary statT (zhT or nyT).
                r-part first (4 tiny mms) so sigmoid_r's wait resolves early."""
                psr, pszn, psx, xs = ps_map[t]
                psr2 = psr[:].rearrange("p c g -> p (c g)")
                pszn2 = pszn[:].rearrange("p c g -> p (c g)")
                rms, zms = [], []
                for i in range(NCH):
                    last = stop and (i == NCH - 1)
                    mr = nc.tensor.matmul(
                        psr2[32 * i:32 * (i + 1), :],
                        statT[32 * i:32 * (i + 1), :],
                        whhr[32 * i:32 * (i + 1), :],
                        start=False, stop=last, skip_group_check=True,
                        tile_position=(32 * i, 32 * i))
                    rms.append(mr)
                for i in range(NCH):
                    last = stop and (i == NCH - 1)
                    mz = nc.tensor.matmul(
                        pszn2[32 * i:32 * (i + 1), :],
                        statT[32 * i:32 * (i + 1), :],
                        whhzn[32 * i:32 * (i + 1), :],
                        start=False, stop=last,
                        skip_group_check=True,
                        tile_position=(32 * i, 32 * i))
                    zms.append(mz)
                for i in range(NCH):
                    for (m_r, m_z) in xs:
                        add_dep_helper(rms[i].ins, m_r.ins, False, "accum order")
                        add_dep_helper(zms[i].ins, m_z.ins, False, "accum order")
                    if after is not None:
                        pr, pz = after[i]
                        add_dep_helper(rms[i].ins, pr.ins, False, "accum order")
                        add_dep_helper(zms[i].ins, pz.ins, False, "accum order")
                return list(zip(rms, zms))

            def emit_step(t):
                psr, pszn, psx, xs = ps_map[t]
                r = wpool.tile([128, NCH, 8], dt.bfloat16, tag="r", name="r")
                z = wpool.tile([128, NCH, 8], dt.bfloat16, tag="z", name="z")
                xpn = wpool.tile([128, NCH, 8], dt.float32, tag="xpn", name="xpn")
                m1 = wpool.tile([128, NCH, 8], dt.float32, tag="m1", name="m1")
                u = wpool.tile([128, NCH, 8], dt.float32, tag="u", name="u")
                n = wpool.tile([128, NCH, 8], dt.bfloat16, tag="n", name="n")
                y = wpool.tile([128, NCH, 8], dt.bfloat16, tag="y", name="y")
                zh = wpool.tile([128, NCH, 8], dt.bfloat16, tag="zh", name="zh")
                ny = wpool.tile([128, NCH, 8], dt.bfloat16, tag="ny", name="ny")

                # ACT: on-chain sigmoid_r first; sigmoid_z and the xpn staging
                # copy are pinned behind chain ops so the static scheduler
                # cannot place them on the critical path.
                s_r = nc.scalar.activation(r[:], psr[:], AF.Sigmoid)
                s_z = nc.scalar.activation(z[:], pszn[:, :, 0:8], AF.Sigmoid)
                add_dep_helper(s_z.ins, s_r.ins, False, "keep sig_z off chain")
                cp = nc.scalar.copy(xpn[:], psx[:])
                if prev_tanh[0] is not None:
                    add_dep_helper(cp.ins, prev_tanh[0].ins, False,
                                   "keep xpn copy off chain")

                # DVE on-chain: m1 = r*ghn (single PSUM read), u = m1 + xpn
                nc.vector.tensor_mul(m1[:], r[:], pszn[:, :, 8:16])
                nc.vector.tensor_add(u[:], m1[:], xpn[:])

                prev_tanh[0] = nc.scalar.activation(n[:], u[:], AF.Tanh)

                # DVE off-chain (fills the u->tanh window): y, zh, zhT
                nc.vector.tensor_scalar(y[:], z[:], -1.0, 1.0, ALU.mult, ALU.add)
                hv = h[:].rearrange("p (c j) -> p c j", c=NCH)
                nc.vector.tensor_mul(zh[:], z[:], hv)
                nc.vector.transpose(zhT[:], zh[:].rearrange("p c j -> p (c j)"))

                # DVE on-chain tail: ny, nyT
                nc.vector.tensor_mul(ny[:], n[:], y[:])
                nc.vector.transpose(nyT[:], ny[:].rearrange("p c j -> p (c j)"))

                # DVE off-chain: h' = ny + zh (for next zh and final decode)
                nc.vector.tensor_add(hv, ny[:], zh[:])

            for t in range(T):
                t2, tp = divmod(t, 2)
                ki, ko = divmod(t2, DMA_T2)
                if ko == 0 and tp == 0:
                    xsb = xpool.tile([128, DMA_T2, NCH, 128], dt.bfloat16,
                                     tag="xsb", name=f"xsb{ki}")
                    nc.sync.dma_start(
                        out=xsb[:],
                        in_=x2_d[:, ki * DMA_T2:(ki + 1) * DMA_T2, :, :],
                    )
                    xsb_map[ki] = xsb
                emit_x(t)
                # W_hh @ zh(t-1): ready early, off the critical chain
                # (at t=0 both stationaries are the zero-memset tiles: exact)
                zh_mms = emit_hmm(t, zhT, stop=False)
                # W_hh @ ny(t-1): the critical-path matmuls
                emit_hmm(t, nyT, stop=True, after=zh_mms)
                emit_step(t)
                ps_map.pop(t - 2, None)

            # decode: out[p, c] = sum_j h * wdec + b_dec
            prod = wpool.tile([128, NCH, 8], dt.float32, tag="prod")
            nc.vector.tensor_mul(
                prod[:], h[:].rearrange("p (c j) -> p c j", c=NCH),
                wdec[:].rearrange("p (c j) -> p c j", c=NCH),
            )
            res = wpool.tile([128, NCH, 1], dt.float32, tag="res")
            nc.vector.tensor_reduce(
                res[:], prod[:], axis=mybir.AxisListType.X, op=mybir.AluOpType.add
            )
            res2 = wpool.tile([128, NCH], dt.float32, tag="res2")
            nc.vector.tensor_scalar_add(
                res2[:], res[:].rearrange("p c one -> p (c one)"), float(b_dec_val))
            nc.sync.dma_start(out=out_d, in_=res2[:])

    nc.compile()
    return nc


def _prep_inputs(x, w_ih, w_hh, b_ih, b_hh, w_dec, b_dec):
    """Returns per-core in_maps list."""
    w_ih = np.asarray(w_ih, np.float32)
    w_hh = np.asarray(w_hh, np.float32)
    b_ih = np.asarray(b_ih, np.float32)
    b_hh = np.asarray(b_hh, np.float32)
    w_dec = np.asarray(w_dec, np.float32)

    # x-projection weights, stationary = x^T [64f, 128s], moving = wih*
    wihr = np.tile(w_ih[0:8].T, (2, 1)).astype(bf16)            # [128, 8]
    wihzn = np.zeros((64, 16), np.float32)
    wihzn[:, 0:8] = w_ih[8:16].T                                # z cols
    wihzn = np.tile(wihzn, (2, 1)).astype(bf16)                 # [128, 16]
    wihn = np.tile(w_ih[16:24].T, (2, 1)).astype(bf16)          # [128, 8]

    # recurrent weights, block-diag over chunks; stationary = (zh|ny)^T
    def blockdiag(wpart, gw):
        # wpart: [gw, 8] rows of w_hh ; returns [128, NCH*gw]
        m = np.zeros((32, NCH, gw), np.float32)
        for c in range(NCH):
            m[c * 8:(c + 1) * 8, c, :] = wpart.T                # [8j, gw]
        m = m.reshape(32, NCH * gw)
        return np.tile(m, (4, 1)).astype(bf16)

    whhr = blockdiag(w_hh[0:8], 8)                              # [128, 32]
    whhzn = blockdiag(np.concatenate([w_hh[8:16], w_hh[16:24]]), 16)  # [128, 64]

    biasr = np.tile(b_ih[0:8] + b_hh[0:8], NCH).reshape(1, 32).astype(bf16)
    biaszn = np.tile(np.concatenate([b_ih[8:16] + b_hh[8:16], b_hh[16:24]]),
                     NCH).reshape(1, 64).astype(bf16)
    biasn = np.tile(b_ih[16:24], NCH).reshape(1, 32).astype(bf16)

    ones = np.ones((1, 128), bf16)
    wdec_b = np.tile(w_dec[0].astype(bf16).astype(np.float32), (128, NCH)).astype(bf16)

    x = np.asarray(x, np.float32)
    in_maps = []
    for core in range(NCORES):
        xc = x[core * BL:(core + 1) * BL]                      # [512, 200, 64]
        tmp = xc.reshape(NCH, 128, T2, 2, 64)                  # ch, s, t2, tp, f
        x2 = np.ascontiguousarray(
            tmp.transpose(3, 4, 2, 0, 1).reshape(128, T2, NCH, 128)
        ).astype(bf16)
        in_maps.append({
            "x2": x2, "wihr": wihr, "wihzn": wihzn, "wihn": wihn,
            "whhr": whhr, "whhzn": whhzn, "biasr": biasr, "biaszn": biaszn,
            "biasn": biasn, "ones": ones, "wdec": wdec_b,
        })
    return in_maps


def kernel(x, w_ih, w_hh, b_ih, b_hh, w_dec, b_dec):
    global LAST_RESULTS
    from concourse import bass_utils

    b_dec_val = float(np.asarray(b_dec, np.float32).reshape(-1)[0])
    nc = _build_program(b_dec_val)
    in_maps = _prep_inputs(x, w_ih, w_hh, b_ih, b_hh, w_dec, b_dec)
    res = bass_utils.run_bass_kernel_spmd(
        nc, in_maps, core_ids=list(range(NCORES)),
        trace=bool(int(os.environ.get("KERNEL_TRACE", "0"))),
    )
    LAST_RESULTS = res
    out = np.empty(B, np.float32)
    for core in range(NCORES):
        o = np.asarray(res.results[core]["out"])               # [128, 4]
        out[core * BL:(core + 1) * BL] = o.T.reshape(-1)
    return out
